# revision 1
# baseline (speedup 1.0000x reference)
"""GCN (2x GCNConv + BN + PReLU, global max pool, 2-layer MLP head) on 8
Trainium2 NeuronCores via Bass/Tile.

Graph-parallel over node shards. Per layer: each core computes
h' = dis * (act @ W) for its 12.5k-row shard (PE), AllGather -> full h';
edge aggregation gathers h'[src] rows with dma_gather (int16 indices, the
source split into <=32500-row chunks) as a padded edge-slot stream
[128 x cols x 64], and segment-reduces by destination with PE matmuls
against on-device one-hot selection matrices (built by DVE is_equal against
an iota row). BN stats via PE ones-matmuls + tiny AllReduce. Global max
pool gathers each core's own out2 rows grouped by graph (single-chunk
dma_gather), reduces max AND min (min covers negative BN scale), and
AllReduce-max combines; BN2+PReLU are applied after pooling (monotone per
channel, choosing max/min by the sign of the BN scale). The readout MLP is
replicated; core 0's output is returned.
"""

import sys

for _p in ("/opt/trn_rl_repo",):
    if _p not in sys.path:
        sys.path.insert(0, _p)

from dataclasses import dataclass, field

import numpy as np

CH_ROWS = 32500  # dma_gather int16 index limit (< 32768 rows per chunk)


@dataclass
class Cfg:
    n_nodes: int = 100000
    n_edges: int = 1250000
    n_graphs: int = 512
    f_in: int = 16
    hid: int = 64
    n_cores: int = 8
    gcols: int = 128          # gather-buffer slot columns per group
    niter: int = 1
    eps: float = 1e-5

    @property
    def shard(self):
        assert self.n_nodes % self.n_cores == 0
        return self.n_nodes // self.n_cores

    @property
    def nt(self):
        return -(-self.shard // 128)

    @property
    def pg(self):
        return -(-self.n_graphs // 128)

    @property
    def nch(self):
        return -(-self.n_nodes // CH_ROWS)


@dataclass
class Layout:
    chunks: list = field(default_factory=list)     # (base, rows)
    groups: list = field(default_factory=list)     # (t0, t1)
    group_cols: list = field(default_factory=list)  # (a, b) abs col range
    calls: list = field(default_factory=list)      # per group: [(j, ca, cn)]
    tcols: list = field(default_factory=list)      # per tile: [(cs, n)]
    ecols: int = 0
    Kp: list = field(default_factory=list)
    poff: list = field(default_factory=list)


def prep(cfg: Cfg, x, edge_index, batch):
    """Relabel nodes and build per-core padded gather indices."""
    N, NC, S, P = cfg.n_nodes, cfg.n_cores, cfg.shard, 128
    src, dst = np.asarray(edge_index[0]), np.asarray(edge_index[1])
    batch = np.asarray(batch)
    indeg = np.bincount(dst, minlength=N).astype(np.int64)

    order = np.argsort(indeg, kind="stable")
    pos = np.arange(N)
    new_id_of_pos = (pos % NC) * S + pos // NC
    new_of_old = np.empty(N, np.int64)
    new_of_old[order] = new_id_of_pos

    # every (graph, core) pair must be non-empty (pool pads repeat a member)
    for _ in range(64):
        old_of_new = np.empty(N, np.int64)
        old_of_new[new_of_old] = np.arange(N)
        gid_new = batch[old_of_new]
        cnt = np.zeros((NC, cfg.n_graphs), np.int64)
        for c in range(NC):
            cnt[c] = np.bincount(gid_new[c * S:(c + 1) * S],
                                 minlength=cfg.n_graphs)
        empt = np.argwhere(cnt == 0)
        if len(empt) == 0:
            break
        c, g = empt[0]
        d = int(np.argmax(cnt[:, g]))
        u_loc = np.where(gid_new[d * S:(d + 1) * S] == g)[0][0]
        gv = int(np.argmax(cnt[c]))
        v_loc = np.where(gid_new[c * S:(c + 1) * S] == gv)[0][0]
        u_old = old_of_new[d * S + u_loc]
        v_old = old_of_new[c * S + v_loc]
        new_of_old[u_old], new_of_old[v_old] = (
            new_of_old[v_old], new_of_old[u_old])
    else:
        raise RuntimeError("could not fix empty (graph,core) pairs")

    old_of_new = np.empty(N, np.int64)
    old_of_new[new_of_old] = np.arange(N)
    gid_new = batch[old_of_new]
    indeg_new = indeg[old_of_new]
    src_new = new_of_old[src]
    dst_new = new_of_old[dst]

    lay = Layout()
    NT, NCH = cfg.nt, cfg.nch
    for j in range(NCH):
        lay.chunks.append((j * CH_ROWS, min(CH_ROWS, N - j * CH_ROWS)))

    def tw(t):
        return min(128, S - t * 128)

    # per (core, tile, chunk) edge counts -> uniform run widths in columns
    c_e = dst_new // S
    loc_e = dst_new - c_e * S
    t_e = loc_e // 128
    j_e = np.minimum(src_new // CH_ROWS, NCH - 1)
    cnt3 = np.zeros((NC, NT, NCH), np.int64)
    np.add.at(cnt3, (c_e, t_e, j_e), 1)
    colw = -(-cnt3.max(axis=0) // 128)          # [NT, NCH] cols per run
    tilecols = colw.sum(axis=1)                  # [NT]
    assert tilecols.max() <= cfg.gcols, (
        f"tile needs {tilecols.max()} cols > gcols={cfg.gcols}")

    # groups of consecutive tiles
    t0 = 0
    while t0 < NT:
        t1, acc = t0, 0
        while t1 < NT and acc + tilecols[t1] <= cfg.gcols:
            acc += tilecols[t1]
            t1 += 1
        lay.groups.append((t0, t1))
        t0 = t1

    # column layout: for each group, for each chunk, runs of its tiles
    runstart = np.zeros((NT, NCH), np.int64)
    lay.tcols = [[] for _ in range(NT)]
    cur = 0
    for (t0, t1) in lay.groups:
        a_g = cur
        calls = []
        for j in range(NCH):
            ca = cur
            for t in range(t0, t1):
                runstart[t, j] = cur
                if colw[t, j] > 0:
                    lay.tcols[t].append((int(cur), int(colw[t, j])))
                cur += int(colw[t, j])
            calls.append((j, int(ca), int(cur - ca)))
        lay.calls.append(calls)
        lay.group_cols.append((int(a_g), int(cur)))
    lay.ecols = int(cur)
    EC = lay.ecols

    # fill per-core index + dstloc arrays
    eidx16 = np.zeros((NC, P, EC * 8), np.int16)
    dstloc = np.full((NC, P, EC), -1.0, np.float32)
    for c in range(NC):
        m = c_e == c
        te, je = t_e[m], j_e[m]
        sn, de = src_new[m], loc_e[m]
        o = np.lexsort((je, te))
        te, je, sn, de = te[o], je[o], sn[o], de[o]
        # run boundaries over sorted (t, j)
        key = te * NCH + je
        starts = np.searchsorted(key, np.arange(NT * NCH))
        ends = np.searchsorted(key, np.arange(NT * NCH) + 1)
        for t in range(NT):
            for j in range(NCH):
                s0, s1 = starts[t * NCH + j], ends[t * NCH + j]
                n = s1 - s0
                if n == 0:
                    continue
                rs = runstart[t, j]
                base = lay.chunks[j][0]
                k = np.arange(n)
                eidx16[c, k % 16, rs * 8 + k // 16] = (sn[s0:s1] - base
                                                       ).astype(np.int16)
                dstloc[c, k % 128, rs + k // 128] = (de[s0:s1] - t * 128
                                                     ).astype(np.float32)
    # replicate the 16-row index block across all 128 partitions
    eidx16 = np.tile(eidx16[:, :16, :], (1, 8, 1))

    degsw = np.ones((NC, P, NT), np.float32)
    inde2 = indeg_new.reshape(NC, S)
    for c in range(NC):
        for t in range(NT):
            w = tw(t)
            degsw[c, :w, t] = inde2[c, t * 128:t * 128 + w] + 1.0

    # pool indices: slot kk -> graph kk%128, member kk//128 (local rows)
    assert S <= 32767, "pool gather needs int16-addressable shard"
    PG = cfg.pg
    for gt in range(PG):
        w = min(128, cfg.n_graphs - gt * 128)
        m = 1
        for c in range(NC):
            m = max(m, int(np.bincount(
                gid_new[c * S:(c + 1) * S],
                minlength=cfg.n_graphs)[gt * 128: gt * 128 + w].max()))
        lay.Kp.append(m)
    lay.poff = np.concatenate([[0], np.cumsum(lay.Kp)]).astype(int).tolist()
    PT = lay.poff[-1]
    pidx16 = np.zeros((NC, P, PT * 8), np.int16)
    for c in range(NC):
        gl = gid_new[c * S:(c + 1) * S]
        po = np.argsort(gl, kind="stable")
        gs = gl[po]
        gstart = np.searchsorted(gs, np.arange(cfg.n_graphs + 1))
        for gt in range(PG):
            w = min(128, cfg.n_graphs - gt * 128)
            Kp, o0 = lay.Kp[gt], lay.poff[gt]
            for p in range(w):
                g = gt * 128 + p
                mem = po[gstart[g]:gstart[g + 1]]
                assert len(mem) > 0
                row = np.full(Kp, mem[0], np.int64)
                row[:len(mem)] = mem[:Kp]
                kk = np.arange(Kp) * 128 + p
                pidx16[c, kk % 16, o0 * 8 + kk // 16] = row.astype(np.int16)
    pidx16 = np.tile(pidx16[:, :16, :], (1, 8, 1))

    x = np.asarray(x, np.float32)
    per_core = []
    for c in range(NC):
        per_core.append(dict(
            xTme=np.ascontiguousarray(x[old_of_new[c * S:(c + 1) * S]].T),
            degsw=np.ascontiguousarray(degsw[c]),
            eidx16=np.ascontiguousarray(eidx16[c]),
            dstloc=np.ascontiguousarray(dstloc[c]),
            pidx16=np.ascontiguousarray(pidx16[c]),
        ))
    return per_core, lay, old_of_new


def build(cfg: Cfg, lay: Layout, debug_taps=False):
    import concourse.bacc as bacc
    import concourse.mybir as mybir
    import concourse.tile as tile

    F32 = mybir.dt.float32
    I16 = mybir.dt.int16
    Alu = mybir.AluOpType
    AX = mybir.AxisListType
    AF = mybir.ActivationFunctionType

    N, S, NT, PG = cfg.n_nodes, cfg.shard, cfg.nt, cfg.pg
    H, F = cfg.hid, cfg.f_in
    G = cfg.n_graphs
    NC = cfg.n_cores
    P = 128
    EC = lay.ecols
    PT = lay.poff[-1]
    PKMAX = max(lay.Kp)
    RG = [list(range(NC))]

    nc = bacc.Bacc("TRN2", target_bir_lowering=False, debug=False,
                   num_devices=NC)

    xTme = nc.dram_tensor("xTme", [F, S], F32, kind="ExternalInput")
    degsw = nc.dram_tensor("degsw", [P, NT], F32, kind="ExternalInput")
    eidx16 = nc.dram_tensor("eidx16", [P, EC * 8], I16, kind="ExternalInput")
    dstloc = nc.dram_tensor("dstloc", [P, EC], F32, kind="ExternalInput")
    pidx16 = nc.dram_tensor("pidx16", [P, PT * 8], I16, kind="ExternalInput")
    iotar = nc.dram_tensor("iotar", [P, P], F32, kind="ExternalInput")
    W1 = nc.dram_tensor("W1", [F, H], F32, kind="ExternalInput")
    W2 = nc.dram_tensor("W2", [H, H], F32, kind="ExternalInput")
    lw1 = nc.dram_tensor("lw1", [H, H], F32, kind="ExternalInput")
    lw2 = nc.dram_tensor("lw2", [H, 1], F32, kind="ExternalInput")
    b1bc = nc.dram_tensor("b1bc", [P, H], F32, kind="ExternalInput")
    b2bc = nc.dram_tensor("b2bc", [P, H], F32, kind="ExternalInput")
    vecs = {}
    for nm in ("bn1w", "bn1b", "bn2w", "bn2b", "bn3w", "bn3b",
               "a1c", "a2c", "a3c", "lb1c"):
        vecs[nm] = nc.dram_tensor(nm, [H, 1], F32, kind="ExternalInput")
    for nm in ("bn4w", "bn4b", "a4c", "lb2c"):
        vecs[nm] = nc.dram_tensor(nm, [1, 1], F32, kind="ExternalInput")
    out_ext = nc.dram_tensor("out", [1, G], F32, kind="ExternalOutput")
    dbg = {}
    if debug_taps:
        for nm, shp in (("dbg_h1p", [S, H]), ("dbg_H1", [N, H]),
                        ("dbg_out1", [S, H]), ("dbg_out2", [S, H]),
                        ("dbg_pool", [2 * PG * 128, H]),
                        ("dbg_st1", [H, 2])):
            dbg[nm] = nc.dram_tensor(nm, shp, F32, kind="ExternalOutput")

    h1p_mine = nc.dram_tensor("h1p_mine", [S, H], F32)
    h2p_mine = nc.dram_tensor("h2p_mine", [S, H], F32)
    out1_mine = nc.dram_tensor("out1_mine", [S, H], F32)
    out2_mine = nc.dram_tensor("out2_mine", [S, H], F32)
    H1 = nc.dram_tensor("H1full", [N, H], F32, addr_space="Shared")
    H2 = nc.dram_tensor("H2full", [N, H], F32, addr_space="Shared")
    st_in = [nc.dram_tensor(f"st{i}_in", [H, 2], F32) for i in (1, 2)]
    st_out = [nc.dram_tensor(f"st{i}_out", [H, 2], F32, addr_space="Shared")
              for i in (1, 2)]
    pool_in = nc.dram_tensor("pool_in", [2 * PG * 128, H], F32)
    pool_out = nc.dram_tensor("pool_out", [2 * PG * 128, H], F32,
                              addr_space="Shared")

    with tile.TileContext(nc) as tc:
        cst = tc.alloc_tile_pool(name="cst", bufs=1)
        psum = tc.alloc_tile_pool(name="psum", bufs=2, space="PSUM")
        spsum = tc.alloc_tile_pool(name="spsum", bufs=1, space="PSUM")
        wrk = tc.alloc_tile_pool(name="wrk", bufs=3)
        gatp = tc.alloc_tile_pool(name="gatp", bufs=2)

        ident = cst.tile([P, P], F32, tag="ident", name="ident")
        from concourse.masks import make_identity
        make_identity(nc, ident[:])
        ones = cst.tile([P, 1], F32, tag="ones", name="ones")
        nc.vector.memset(ones[:], 1.0)
        eps_sb = cst.tile([P, 1], F32, tag="eps_sb", name="eps_sb")
        nc.vector.memset(eps_sb[:], cfg.eps)
        W1sb = cst.tile([F, H], F32, tag="W1sb", name="W1sb")
        nc.sync.dma_start(out=W1sb[:], in_=W1[:, :])
        W2sb = cst.tile([H, H], F32, tag="W2sb", name="W2sb")
        nc.sync.dma_start(out=W2sb[:], in_=W2[:, :])
        lw1sb = cst.tile([H, H], F32, tag="lw1sb", name="lw1sb")
        nc.sync.dma_start(out=lw1sb[:], in_=lw1[:, :])
        lw2sb = cst.tile([H, 1], F32, tag="lw2sb", name="lw2sb")
        nc.sync.dma_start(out=lw2sb[:], in_=lw2[:, :])
        b1sb = cst.tile([P, H], F32, tag="b1sb", name="b1sb")
        nc.sync.dma_start(out=b1sb[:], in_=b1bc[:, :])
        b2sb = cst.tile([P, H], F32, tag="b2sb", name="b2sb")
        nc.sync.dma_start(out=b2sb[:], in_=b2bc[:, :])
        iot = cst.tile([P, P], F32, tag="iot", name="iot")
        nc.sync.dma_start(out=iot[:], in_=iotar[:, :])
        vsb = {}
        for nm, t in vecs.items():
            sh = [H, 1] if t.shape[0] == H else [1, 1]
            vsb[nm] = cst.tile(sh, F32, tag=f"v_{nm}", name=f"v_{nm}")
            nc.sync.dma_start(out=vsb[nm][:], in_=t[:, :])
        eix = cst.tile([P, EC * 8], I16, tag="eix", name="eix")
        nc.sync.dma_start(out=eix[:], in_=eidx16[:, :])
        dlc = cst.tile([P, EC], F32, tag="dlc", name="dlc")
        nc.sync.dma_start(out=dlc[:], in_=dstloc[:, :])
        pix = cst.tile([P, PT * 8], I16, tag="pix", name="pix")
        nc.sync.dma_start(out=pix[:], in_=pidx16[:, :])
        dsw = cst.tile([P, NT], F32, tag="dsw", name="dsw")
        nc.sync.dma_start(out=dsw[:], in_=degsw[:, :])
        dis = cst.tile([P, NT], F32, tag="dis", name="dis")
        nc.vector.reciprocal(dis[:], dsw[:])
        nc.scalar.sqrt(dis[:], dis[:])

        zt = cst.tile([P, H], F32, tag="zt", name="zt")
        nc.vector.memset(zt[:], 0.0)
        for j in range(2 * PG):
            nc.sync.dma_start(out=pool_in[j * 128:(j + 1) * 128, :],
                              in_=zt[:, :])

        def tw(t):
            return min(128, S - t * 128)

        def affine(out_ap, in_ap, sc, sh):
            nc.vector.tensor_scalar(out_ap, in_ap, sc, sh, Alu.mult, Alu.add)

        def prelu(ap, a_ap, tag):
            tmp = wrk.tile(list(ap.shape), F32, tag=tag, name=tag)
            nc.vector.tensor_scalar(tmp[:], ap, a_ap, None, Alu.mult)
            nc.vector.tensor_tensor(out=ap, in0=ap, in1=tmp[:], op=Alu.max)

        def mm_phase(src_dram, dst_dram, Wsb, kdim, scale_sb, shift_sb, a_sb,
                     layer):
            for t in range(NT):
                w = tw(t)
                if layer == 1:
                    rhs = wrk.tile([F, P], F32, tag="mm_rhs1", name="mm_rhs1")
                    nc.sync.dma_start(out=rhs[:, :w],
                                      in_=src_dram[:, t * 128:t * 128 + w])
                    rhs_ap = rhs[:F, :w]
                else:
                    o1 = wrk.tile([P, H], F32, tag="mm_in2", name="mm_in2")
                    nc.sync.dma_start(out=o1[:w, :],
                                      in_=src_dram[t * 128:t * 128 + w, :])
                    pst = psum.tile([H, P], F32, tag="mm_pst", name="mm_pst")
                    nc.tensor.transpose(pst[:H, :w], o1[:w, :H],
                                        ident[:w, :w])
                    act = wrk.tile([H, P], F32, tag="mm_act", name="mm_act")
                    affine(act[:H, :w], pst[:H, :w], scale_sb, shift_sb)
                    prelu(act[:H, :w], a_sb, "mm_prelu")
                    rhs_ap = act[:H, :w]
                psm = psum.tile([H, P], F32, tag="mm_psm", name="mm_psm")
                nc.tensor.matmul(psm[:H, :w], lhsT=Wsb[:kdim, :H],
                                 rhs=rhs_ap, start=True, stop=True)
                hT = wrk.tile([H, P], F32, tag="mm_hT", name="mm_hT")
                nc.scalar.copy(hT[:H, :w], psm[:H, :w])
                ps2 = psum.tile([P, H], F32, tag="mm_ps2", name="mm_ps2")
                nc.tensor.transpose(ps2[:w, :H], hT[:H, :w], ident[:H, :H])
                stg = wrk.tile([P, H], F32, tag="mm_stg", name="mm_stg")
                nc.vector.tensor_scalar(stg[:w, :], ps2[:w, :H],
                                        dis[:w, t:t + 1], None, Alu.mult)
                nc.sync.dma_start(out=dst_dram[t * 128:t * 128 + w, :],
                                  in_=stg[:w, :])

        def agg_phase(Hfull, hp_mine, outx_mine, bsb, st_in_t, st_out_t,
                      scale_t, shift_t, bnw, bnb):
            s1 = spsum.tile([H, 1], F32, tag="s1", name="s1")
            s2 = spsum.tile([H, 1], F32, tag="s2", name="s2")
            for gi, (t0, t1) in enumerate(lay.groups):
                a_g, b_g = lay.group_cols[gi]
                gat = gatp.tile([P, cfg.gcols * H], F32, tag="gat",
                                name="gat")
                for (j, ca, cn) in lay.calls[gi]:
                    if cn == 0:
                        continue
                    base, rows = lay.chunks[j]
                    nidx = cn * 128
                    o0 = (ca - a_g) * H
                    nc.gpsimd.dma_gather(
                        gat[:, o0:o0 + cn * H].rearrange(
                            "p (k f) -> p k f", f=H),
                        Hfull[base:base + rows, :],
                        eix[:, ca * 8:(ca + cn) * 8],
                        nidx, nidx, H)
                for t in range(t0, t1):
                    w = tw(t)
                    runs = lay.tcols[t]
                    total = sum(n for _, n in runs)
                    ps_agg = psum.tile([P, H], F32, tag="agg_ps",
                                       name="agg_ps")
                    done = 0
                    for (cs, n) in runs:
                        c0 = cs
                        while c0 < cs + n:
                            nb = min(4, cs + n - c0)
                            S4 = wrk.tile([P, 4 * P], F32, tag="S4",
                                          name="S4")
                            s4v = S4[:, :nb * P].rearrange(
                                "p (k j) -> p k j", j=P)
                            nc.vector.tensor_tensor(
                                out=s4v,
                                in0=iot[:].rearrange(
                                    "p (o j) -> p o j", o=1
                                ).to_broadcast([P, nb, P]),
                                in1=dlc[:, c0:c0 + nb].rearrange(
                                    "p (k o) -> p k o", o=1
                                ).to_broadcast([P, nb, P]),
                                op=Alu.is_equal)
                            for k in range(nb):
                                cc = c0 + k
                                nc.tensor.matmul(
                                    ps_agg[:w, :],
                                    lhsT=S4[:, k * P:k * P + P][:, :w],
                                    rhs=gat[:, (cc - a_g) * H:
                                            (cc - a_g) * H + H],
                                    start=(done == 0),
                                    stop=(done == total - 1),
                                    skip_group_check=True)
                                done += 1
                            c0 += nb
                    slf = wrk.tile([P, H], F32, tag="slf", name="slf")
                    nc.sync.dma_start(out=slf[:w, :],
                                      in_=hp_mine[t * 128:t * 128 + w, :])
                    agg = wrk.tile([P, H], F32, tag="agg", name="agg")
                    nc.vector.tensor_tensor(out=agg[:w, :],
                                            in0=ps_agg[:w, :],
                                            in1=slf[:w, :], op=Alu.add)
                    nc.vector.tensor_scalar(agg[:w, :], agg[:w, :],
                                            dis[:w, t:t + 1], None, Alu.mult)
                    o = wrk.tile([P, H], F32, tag="oX", name="oX")
                    nc.vector.tensor_tensor(out=o[:w, :], in0=agg[:w, :],
                                            in1=bsb[:w, :], op=Alu.add)
                    nc.sync.dma_start(out=outx_mine[t * 128:t * 128 + w, :],
                                      in_=o[:w, :])
                    sq = wrk.tile([P, H], F32, tag="sq", name="sq")
                    nc.scalar.square(sq[:w, :], o[:w, :])
                    nc.tensor.matmul(s1[:, :], lhsT=o[:w, :H],
                                     rhs=ones[:w, :1], start=(t == 0),
                                     stop=(t == NT - 1),
                                     skip_group_check=True)
                    nc.tensor.matmul(s2[:, :], lhsT=sq[:w, :H],
                                     rhs=ones[:w, :1], start=(t == 0),
                                     stop=(t == NT - 1),
                                     skip_group_check=True)
            st = wrk.tile([H, 2], F32, tag="stx", name="stx")
            nc.vector.tensor_copy(out=st[:, 0:1], in_=s1[:, :])
            nc.vector.tensor_copy(out=st[:, 1:2], in_=s2[:, :])
            nc.sync.dma_start(out=st_in_t[:, :], in_=st[:, :])
            nc.gpsimd.collective_compute(
                "AllReduce", Alu.add, replica_groups=RG,
                ins=[st_in_t[:, :]], outs=[st_out_t[:, :]])
            str_ = wrk.tile([H, 2], F32, tag="str", name="str")
            nc.sync.dma_start(out=str_[:, :], in_=st_out_t[:, :])
            mean = wrk.tile([H, 1], F32, tag="mean", name="mean")
            nc.vector.tensor_scalar(mean[:], str_[:, 0:1], 1.0 / N, None,
                                    Alu.mult)
            ex2 = wrk.tile([H, 1], F32, tag="ex2", name="ex2")
            nc.vector.tensor_scalar(ex2[:], str_[:, 1:2], 1.0 / N, None,
                                    Alu.mult)
            var = wrk.tile([H, 1], F32, tag="var", name="var")
            nc.vector.tensor_tensor(out=var[:], in0=mean[:], in1=mean[:],
                                    op=Alu.mult)
            nc.vector.tensor_tensor(out=var[:], in0=ex2[:], in1=var[:],
                                    op=Alu.subtract)
            sd = wrk.tile([H, 1], F32, tag="sd", name="sd")
            nc.scalar.activation(sd[:], var[:], AF.Sqrt, bias=eps_sb[:H, :])
            nc.vector.reciprocal(sd[:], sd[:])
            nc.vector.tensor_tensor(out=scale_t[:], in0=bnw[:], in1=sd[:],
                                    op=Alu.mult)
            nc.vector.tensor_tensor(out=shift_t[:], in0=mean[:],
                                    in1=scale_t[:], op=Alu.mult)
            nc.vector.tensor_tensor(out=shift_t[:], in0=bnb[:],
                                    in1=shift_t[:], op=Alu.subtract)

        def stats_T(z_ap, n_count, bnw, bnb, scale_t, shift_t, pdim):
            s1 = wrk.tile([pdim, 1], F32, tag="s1T", name="s1T")
            nc.vector.tensor_reduce(s1[:], z_ap, AX.X, Alu.add)
            sq = wrk.tile([pdim, z_ap.shape[-1]], F32, tag="sqT", name="sqT")
            nc.scalar.square(sq[:], z_ap)
            s2 = wrk.tile([pdim, 1], F32, tag="s2T", name="s2T")
            nc.vector.tensor_reduce(s2[:], sq[:], AX.X, Alu.add)
            nc.vector.tensor_scalar(s1[:], s1[:], 1.0 / n_count, None,
                                    Alu.mult)
            nc.vector.tensor_scalar(s2[:], s2[:], 1.0 / n_count, None,
                                    Alu.mult)
            v = wrk.tile([pdim, 1], F32, tag="vT", name="vT")
            nc.vector.tensor_tensor(out=v[:], in0=s1[:], in1=s1[:],
                                    op=Alu.mult)
            nc.vector.tensor_tensor(out=v[:], in0=s2[:], in1=v[:],
                                    op=Alu.subtract)
            nc.scalar.activation(v[:], v[:], AF.Sqrt, bias=eps_sb[:pdim, :])
            nc.vector.reciprocal(v[:], v[:])
            nc.vector.tensor_tensor(out=scale_t[:], in0=bnw[:], in1=v[:],
                                    op=Alu.mult)
            nc.vector.tensor_tensor(out=shift_t[:], in0=s1[:],
                                    in1=scale_t[:], op=Alu.mult)
            nc.vector.tensor_tensor(out=shift_t[:], in0=bnb[:],
                                    in1=shift_t[:], op=Alu.subtract)

        scale1 = cst.tile([H, 1], F32, tag="scale1", name="scale1")
        shift1 = cst.tile([H, 1], F32, tag="shift1", name="shift1")
        scale2 = cst.tile([H, 1], F32, tag="scale2", name="scale2")
        shift2 = cst.tile([H, 1], F32, tag="shift2", name="shift2")

        for _ in range(cfg.niter):
            mm_phase(xTme, h1p_mine, W1sb, F, None, None, None, layer=1)
            nc.gpsimd.collective_compute(
                "AllGather", Alu.bypass, replica_groups=RG,
                ins=[h1p_mine[:, :]], outs=[H1[:, :]])
            agg_phase(H1, h1p_mine, out1_mine, b1sb, st_in[0], st_out[0],
                      scale1, shift1, vsb["bn1w"], vsb["bn1b"])
            mm_phase(out1_mine, h2p_mine, W2sb, H, scale1, shift1,
                     vsb["a1c"], layer=2)
            nc.gpsimd.collective_compute(
                "AllGather", Alu.bypass, replica_groups=RG,
                ins=[h2p_mine[:, :]], outs=[H2[:, :]])
            agg_phase(H2, h2p_mine, out2_mine, b2sb, st_in[1], st_out[1],
                      scale2, shift2, vsb["bn2w"], vsb["bn2b"])
            # pool: per-graph max/min of out2 over own shard
            for gt in range(PG):
                wg = min(128, G - gt * 128)
                Kp = lay.Kp[gt]
                o0 = lay.poff[gt]
                pg_t = gatp.tile([P, PKMAX * H], F32, tag="pgat",
                                 name="pgat")
                nidx = Kp * 128
                nc.gpsimd.dma_gather(
                    pg_t[:, :Kp * H].rearrange("p (k f) -> p k f", f=H),
                    out2_mine[:, :],
                    pix[:, o0 * 8:(o0 + Kp) * 8],
                    nidx, nidx, H)
                view = pg_t[:, :Kp * H].rearrange("p (k f) -> p f k", f=H)
                mx = wrk.tile([P, H], F32, tag="pmx", name="pmx")
                nc.vector.tensor_reduce(mx[:wg, :], view[:wg], AX.X, Alu.max)
                mn = wrk.tile([P, H], F32, tag="pmn", name="pmn")
                nc.vector.tensor_reduce(mn[:wg, :], view[:wg], AX.X, Alu.min)
                nc.vector.tensor_scalar(mn[:wg, :], mn[:wg, :], -1.0, None,
                                        Alu.mult)
                nc.sync.dma_start(
                    out=pool_in[gt * 128:gt * 128 + wg, :], in_=mx[:wg, :])
                nc.sync.dma_start(
                    out=pool_in[PG * 128 + gt * 128:
                                PG * 128 + gt * 128 + wg, :],
                    in_=mn[:wg, :])
            nc.gpsimd.collective_compute(
                "AllReduce", Alu.max, replica_groups=RG,
                ins=[pool_in[:, :]], outs=[pool_out[:, :]])
            # readout (replicated)
            GP = PG * 128
            mmaxT = wrk.tile([H, GP], F32, tag="mmaxT", name="mmaxT")
            mminT = wrk.tile([H, GP], F32, tag="mminT", name="mminT")
            for j in range(2 * PG):
                pt = wrk.tile([P, H], F32, tag="ro_in", name="ro_in")
                nc.sync.dma_start(out=pt[:, :],
                                  in_=pool_out[j * 128:(j + 1) * 128, :])
                pstT = psum.tile([H, P], F32, tag="mm_pst", name="ro_pst")
                nc.tensor.transpose(pstT[:H, :], pt[:, :H], ident[:, :])
                dstT = mmaxT if j < PG else mminT
                off = (j % PG) * 128
                nc.scalar.copy(dstT[:, off:off + 128], pstT[:H, :])
            nc.vector.tensor_scalar(mminT[:], mminT[:], -1.0, None, Alu.mult)
            mask = wrk.tile([H, 1], F32, tag="mask", name="mask")
            nc.vector.tensor_scalar(mask[:], scale2[:], 0.0, None, Alu.is_gt)
            XT = wrk.tile([H, GP], F32, tag="XT", name="XT")
            nc.vector.tensor_tensor(out=XT[:], in0=mmaxT[:], in1=mminT[:],
                                    op=Alu.subtract)
            nc.vector.tensor_scalar(XT[:], XT[:], mask[:], None, Alu.mult)
            nc.vector.tensor_tensor(out=XT[:], in0=XT[:], in1=mminT[:],
                                    op=Alu.add)
            affine(XT[:], XT[:], scale2[:], shift2[:])
            prelu(XT[:], vsb["a2c"][:], "ro_prelu2")
            gT = XT[:, :G]
            ps_z1 = psum.tile([H, G], F32, tag="mm_psm", name="ro_z1")
            nc.tensor.matmul(ps_z1[:, :], lhsT=lw1sb[:H, :H], rhs=gT,
                             start=True, stop=True)
            z1 = wrk.tile([H, G], F32, tag="z1", name="z1")
            nc.vector.tensor_scalar(z1[:], ps_z1[:, :], vsb["lb1c"][:], None,
                                    Alu.add)
            scale3 = wrk.tile([H, 1], F32, tag="scale3", name="scale3")
            shift3 = wrk.tile([H, 1], F32, tag="shift3", name="shift3")
            stats_T(z1[:], G, vsb["bn3w"], vsb["bn3b"], scale3, shift3, H)
            affine(z1[:], z1[:], scale3[:], shift3[:])
            prelu(z1[:], vsb["a3c"][:], "ro_prelu3")
            ps_z2 = psum.tile([1, G], F32, tag="mm_ps2", name="ro_z2")
            nc.tensor.matmul(ps_z2[:, :], lhsT=lw2sb[:H, :1], rhs=z1[:],
                             start=True, stop=True)
            z2 = wrk.tile([1, G], F32, tag="z2", name="z2")
            nc.vector.tensor_scalar(z2[:], ps_z2[:, :], vsb["lb2c"][:], None,
                                    Alu.add)
            scale4 = wrk.tile([1, 1], F32, tag="scale4", name="scale4")
            shift4 = wrk.tile([1, 1], F32, tag="shift4", name="shift4")
            stats_T(z2[:], G, vsb["bn4w"], vsb["bn4b"], scale4, shift4, 1)
            affine(z2[:], z2[:], scale4[:], shift4[:])
            prelu(z2[:], vsb["a4c"][:], "ro_prelu4")
            nc.sync.dma_start(out=out_ext[0:1, :], in_=z2[0:1, :])
            if debug_taps:
                def tap(dstt, srct, nrows):
                    for r0 in range(0, nrows, 128):
                        rw = min(128, nrows - r0)
                        bt = wrk.tile([128, H], F32, tag="tapbt",
                                      name="tapbt")
                        nc.sync.dma_start(out=bt[:rw, :],
                                          in_=srct[r0:r0 + rw, :])
                        nc.sync.dma_start(out=dstt[r0:r0 + rw, :],
                                          in_=bt[:rw, :])
                tap(dbg["dbg_h1p"], h1p_mine, S)
                tap(dbg["dbg_H1"], H1, N)
                tap(dbg["dbg_out1"], out1_mine, S)
                tap(dbg["dbg_out2"], out2_mine, S)
                tap(dbg["dbg_pool"], pool_out, 2 * PG * 128)
                bt2 = wrk.tile([H, 2], F32, tag="tapbt2", name="tapbt2")
                nc.sync.dma_start(out=bt2[:, :], in_=st_out[0][:, :])
                nc.sync.dma_start(out=dbg["dbg_st1"][:, :], in_=bt2[:, :])

        for pool in (gatp, wrk, spsum, psum, cst):
            pool.release()

    nc.compile()
    return nc


def make_in_maps(cfg: Cfg, per_core, inputs):
    H = cfg.hid
    f32 = np.float32
    col = lambda v: np.asarray(v, f32).reshape(-1, 1)
    iotar = np.broadcast_to(np.arange(128, dtype=f32), (128, 128)).copy()
    shared = dict(
        W1=np.asarray(inputs["W1"], f32), W2=np.asarray(inputs["W2"], f32),
        lw1=np.asarray(inputs["lw1"], f32), lw2=np.asarray(inputs["lw2"], f32),
        b1bc=np.broadcast_to(np.asarray(inputs["b1"], f32), (128, H)).copy(),
        b2bc=np.broadcast_to(np.asarray(inputs["b2"], f32), (128, H)).copy(),
        bn1w=col(inputs["bn1_w"]), bn1b=col(inputs["bn1_b"]),
        bn2w=col(inputs["bn2_w"]), bn2b=col(inputs["bn2_b"]),
        bn3w=col(inputs["bn3_w"]), bn3b=col(inputs["bn3_b"]),
        bn4w=col(inputs["bn4_w"]), bn4b=col(inputs["bn4_b"]),
        a1c=np.full((H, 1), float(np.asarray(inputs["a1"]).ravel()[0]), f32),
        a2c=np.full((H, 1), float(np.asarray(inputs["a2"]).ravel()[0]), f32),
        a3c=np.full((H, 1), float(np.asarray(inputs["a3"]).ravel()[0]), f32),
        a4c=np.full((1, 1), float(np.asarray(inputs["a4"]).ravel()[0]), f32),
        lb1c=col(inputs["lb1"]), lb2c=col(inputs["lb2"]),
        iotar=iotar,
    )
    return [dict(shared, **per_core[c]) for c in range(cfg.n_cores)]


_BUILD_CACHE = {}


def run(cfg: Cfg, inputs, time_it=False):
    from concourse.bass_utils import run_bass_kernel_spmd

    per_core, lay, _ = prep(cfg, inputs["x"], inputs["edge_index"],
                            inputs["batch"])
    key = (cfg.n_nodes, cfg.n_edges, cfg.niter, lay.ecols, tuple(lay.Kp))
    if key not in _BUILD_CACHE:
        _BUILD_CACHE[key] = build(cfg, lay)
    nc = _BUILD_CACHE[key]
    in_maps = make_in_maps(cfg, per_core, inputs)
    import time
    t0 = time.perf_counter()
    res = run_bass_kernel_spmd(nc, in_maps, list(range(cfg.n_cores)))
    t1 = time.perf_counter()
    out = res.results[0]["out"].reshape(cfg.n_graphs, 1)
    if time_it:
        return out, (t1 - t0)
    return out


def kernel(**inputs):
    cfg = Cfg()
    return run(cfg, inputs).astype(np.float32)
